# revision 1
# baseline (speedup 1.0000x reference)
"""GAT-style bipartite graph attention layer (nn_BiGraphContrastLayer) on 8 trn2 cores.

Strategy (dst-sharded SPMD, one shared program):
  - Every core computes zel = x @ [W | W@Al | W@Ar] for all N nodes (replicated;
    bf16 matmul, fp32 accum) and writes a per-node row table
    zel_tab[n] = [z(512) | el(8) | er(8) | pad] (bf16, 640 elems = 1280B) to DRAM.
  - Each core owns 1250 dst nodes.  Their incoming edges (+ self loops), sorted
    by dst and grouped into 10 dst tiles of 128, are gathered per edge from
    zel_tab via SWDGE dma_gather (src row: 1280B; dst el/er tail: 256B).
  - v = exp(leaky_relu(el_src + er_dst)) per edge/head; messages msg = v * z_src
    (DVE, per-head broadcast); per-dst-tile segment sums via one-hot selection
    matmuls on the PE accumulating in PSUM: out_tile = SelT.T @ msg and
    s_tile = SelT.T @ v.  Final: out/s + bias.
  No inter-core communication; host concatenates the 8 dst slices.
"""
import os

import numpy as np
import ml_dtypes

import concourse.bacc as bacc
import concourse.bass as bass
import concourse.mybir as mybir
import concourse.tile as tile

BF = ml_dtypes.bfloat16
F32 = np.float32

NS, ND, E, DIN, H, DH = 10000, 10000, 320000, 512, 8, 64
NEG = 0.2
NCORES = 8
DPC = ND // NCORES          # 1250 dst nodes per core
N = NS + ND
NPAD = 20480                # node count padded to 160 tiles of 128
ROW = 640                   # zel row elems: z(512) | el(8) | er(8) | pad(112)
NTILES = (DPC + 127) // 128  # 10 dst tiles per core
PANEL = 2048                # phase-1 node panel (16 subtiles of 128)


# ----------------------------------------------------------------- host prep
def _wrap_idx(idx):
    """dma_gather index layout: idx i -> [i % 16, i // 16], replicated 8x."""
    k = len(idx)
    w = np.zeros((16, k // 16), np.int16)
    w[np.arange(k) % 16, np.arange(k) // 16] = idx
    return np.tile(w, (8, 1))


def _host_prep(x_src, x_dst, edge_src, edge_dst, W, attn_l, attn_r, bias):
    x = np.concatenate([x_src, x_dst], 0).astype(F32)       # [N, 512]
    xT = np.zeros((DIN, NPAD), BF)
    xT[:, :N] = x.T
    Al = np.zeros((DIN, H), F32)
    Ar = np.zeros((DIN, H), F32)
    for h in range(H):
        Al[h * DH:(h + 1) * DH, h] = attn_l[h]
        Ar[h * DH:(h + 1) * DH, h] = attn_r[h]
    Wext = np.concatenate([W, W @ Al, W @ Ar], 1).astype(BF)  # [512, 528]
    bias_rep = np.tile(bias[None, :].astype(F32), (128, 1))   # [128, 512]

    # per-(core, dst tile) edge lists, sorted by local dst
    edge_src = edge_src.astype(np.int64)
    edge_dst = edge_dst.astype(np.int64)
    tlists = [[None] * NTILES for _ in range(NCORES)]
    kmax = 0
    for c in range(NCORES):
        d0 = c * DPC
        m = (edge_dst >= d0) & (edge_dst < d0 + DPC)
        es = np.concatenate([edge_src[m],
                             NS + d0 + np.arange(DPC, dtype=np.int64)])
        ed = np.concatenate([edge_dst[m] - d0, np.arange(DPC, dtype=np.int64)])
        order = np.argsort(ed, kind="stable")
        es, ed = es[order], ed[order]
        for t in range(NTILES):
            sel = (ed >= t * 128) & (ed < (t + 1) * 128)
            tlists[c][t] = (es[sel], ed[sel] - t * 128)
            kmax = max(kmax, int(sel.sum()))
    k_tile = ((kmax + 127) // 128) * 128
    nch = k_tile // 128

    per_core = []
    for c in range(NCORES):
        zidx = np.zeros((128, NTILES * k_tile // 16), np.int16)
        selT = np.zeros((128, NTILES * nch * 128), BF)
        selD = np.zeros((128, NTILES * nch * 128), BF)
        for t in range(NTILES):
            es, edl = tlists[c][t]
            k = len(es)
            src = np.zeros(k_tile, np.int64)
            src[:k] = es
            s16 = slice(t * k_tile // 16, (t + 1) * k_tile // 16)
            zidx[:, s16] = _wrap_idx(src)
            dstl = np.full(k_tile, -1, np.int64)
            dstl[:k] = edl
            for ch in range(nch):
                dl = dstl[ch * 128:(ch + 1) * 128]
                sm = np.zeros((128, 128), F32)
                valid = dl >= 0
                sm[np.arange(128)[valid], dl[valid]] = 1.0
                j = (t * nch + ch) * 128
                selT[:, j:j + 128] = sm.astype(BF)
                selD[:, j:j + 128] = sm.T.astype(BF)
        eridx = _wrap_idx(NS + c * DPC + np.arange(NTILES * 128, dtype=np.int64))
        per_core.append(dict(selT=selT, selD=selD, zidx=zidx, eridx=eridx))

    shared = dict(xT=xT, Wext=Wext, bias_rep=bias_rep)
    return shared, per_core, k_tile, nch


# ------------------------------------------------------------- bass program
def _build_nc(k_tile, nch):
    nc = bacc.Bacc("TRN2", target_bir_lowering=False, debug=False)
    dt = mybir.dt

    xT_d = nc.dram_tensor("xT", [DIN, NPAD], dt.bfloat16, kind="ExternalInput")
    W_d = nc.dram_tensor("Wext", [DIN, 528], dt.bfloat16, kind="ExternalInput")
    bias_d = nc.dram_tensor("bias_rep", [128, 512], dt.float32, kind="ExternalInput")
    selT_d = nc.dram_tensor("selT", [128, NTILES * nch * 128], dt.bfloat16,
                            kind="ExternalInput")
    selD_d = nc.dram_tensor("selD", [128, NTILES * nch * 128], dt.bfloat16,
                            kind="ExternalInput")
    zidx_d = nc.dram_tensor("zidx", [128, NTILES * k_tile // 16], dt.int16,
                            kind="ExternalInput")
    eridx_d = nc.dram_tensor("eridx", [128, NTILES * 128 // 16], dt.int16,
                             kind="ExternalInput")
    out_d = nc.dram_tensor("out", [NTILES * 128, 512], dt.float32,
                           kind="ExternalOutput")
    zel_d = nc.dram_tensor("zel_tab", [NPAD, ROW], dt.bfloat16)

    with tile.TileContext(nc) as tc:
        # ---- constants resident in SBUF
        with tc.tile_pool(name="const", bufs=1) as cpool:
            wsb = cpool.tile([128, 4 * 528], dt.bfloat16)
            for k in range(4):
                nc.sync.dma_start(wsb[:, k * 528:(k + 1) * 528],
                                  W_d[k * 128:(k + 1) * 128, :])
            bias_sb = cpool.tile([128, 512], dt.float32)
            nc.sync.dma_start(bias_sb[:], bias_d[:])
            zidx_sb = cpool.tile([128, NTILES * k_tile // 16], dt.int16)
            nc.sync.dma_start(zidx_sb[:], zidx_d[:])
            eridx_sb = cpool.tile([128, NTILES * 128 // 16], dt.int16)
            nc.sync.dma_start(eridx_sb[:], eridx_d[:])

            # ---- phase 1: zel_tab = [x@W | x@Wl | x@Wr] for all nodes
            with (
                tc.tile_pool(name="xp", bufs=2) as xpool,
                tc.tile_pool(name="zel", bufs=3) as zpool,
                tc.tile_pool(name="p1", bufs=2, space="PSUM") as p1pool,
                tc.tile_pool(name="p1b", bufs=2, space="PSUM") as p1bpool,
            ):
                for p in range(NPAD // PANEL):
                    xp = xpool.tile([128, 4 * PANEL], dt.bfloat16)
                    for k in range(4):
                        nc.sync.dma_start(
                            xp[:, k * PANEL:(k + 1) * PANEL],
                            xT_d[k * 128:(k + 1) * 128,
                                 p * PANEL:(p + 1) * PANEL])
                    for m in range(PANEL // 128):
                        zps = p1pool.tile([128, 512], dt.float32, space="PSUM")
                        lps = p1bpool.tile([128, 16], dt.float32, space="PSUM")
                        for k in range(4):
                            lhsT = xp[:, k * PANEL + m * 128:
                                      k * PANEL + (m + 1) * 128]
                            nc.tensor.matmul(zps[:], lhsT,
                                             wsb[:, k * 528:k * 528 + 512],
                                             start=(k == 0), stop=(k == 3))
                            nc.tensor.matmul(lps[:], lhsT,
                                             wsb[:, k * 528 + 512:(k + 1) * 528],
                                             start=(k == 0), stop=(k == 3))
                        zel_sb = zpool.tile([128, ROW], dt.bfloat16)
                        nc.vector.tensor_copy(zel_sb[:, 0:512], zps[:])
                        nc.vector.tensor_copy(zel_sb[:, 512:528], lps[:])
                        nc.gpsimd.memset(zel_sb[:, 528:ROW], 0)
                        row0 = (p * (PANEL // 128) + m) * 128
                        nc.sync.dma_start(zel_d[row0:row0 + 128, :], zel_sb[:])

            # all phase-1 zel_tab writes must land before gathers read it
            tc.strict_bb_all_engine_barrier()

            # ---- phase 2: per dst tile gather + attention + aggregation
            with (
                tc.tile_pool(name="zg", bufs=3) as zgpool,
                tc.tile_pool(name="era", bufs=1) as erapool,
                tc.tile_pool(name="sel", bufs=3) as selpool,
                tc.tile_pool(name="sc", bufs=3) as scpool,
                tc.tile_pool(name="eo", bufs=2) as eopool,
                tc.tile_pool(name="p2", bufs=3, space="PSUM") as p2pool,
                tc.tile_pool(name="p2b", bufs=3, space="PSUM") as p2bpool,
                tc.tile_pool(name="p2c", bufs=2, space="PSUM") as p2cpool,
            ):
                # er (and el) of this core's dst nodes: one small gather
                era = erapool.tile([128, NTILES, 128], dt.bfloat16)
                nc.gpsimd.dma_gather(
                    era[:], zel_d[:, 512:640], eridx_sb[:],
                    num_idxs=NTILES * 128, num_idxs_reg=NTILES * 128,
                    elem_size=128, elem_step=ROW, single_packet=False)

                for t in range(NTILES):
                    zg = zgpool.tile([128, nch, ROW], dt.bfloat16)
                    sel = selpool.tile([128, nch * 128], dt.bfloat16)
                    nc.sync.dma_start(
                        sel[:], selT_d[:, t * nch * 128:(t + 1) * nch * 128])
                    seld = selpool.tile([128, nch * 128], dt.bfloat16,
                                        tag="seld")
                    nc.sync.dma_start(
                        seld[:], selD_d[:, t * nch * 128:(t + 1) * nch * 128])

                    # er_dst broadcast to edges via Sel matmuls — all chunks
                    # packed into one PSUM bank.
                    lt = scpool.tile([128, nch, 8], dt.float32, tag="lt")
                    pe_er = p2cpool.tile([128, nch, 8], dt.float32,
                                         space="PSUM")
                    for ch in range(nch):
                        nc.tensor.matmul(pe_er[:, ch, :],
                                         seld[:, ch * 128:(ch + 1) * 128],
                                         era[:, t, 8:16],
                                         start=True, stop=True,
                                         skip_group_check=True)
                    i16 = slice(t * k_tile // 16, (t + 1) * k_tile // 16)
                    nc.gpsimd.dma_gather(
                        zg[:], zel_d[:], zidx_sb[:, i16],
                        num_idxs=k_tile, num_idxs_reg=k_tile, elem_size=ROW,
                        single_packet=False)
                    nc.vector.tensor_tensor(
                        lt[:], zg[:, :, 512:520], pe_er[:],
                        op=mybir.AluOpType.add)
                    nc.vector.scalar_tensor_tensor(
                        lt[:], lt[:], NEG, lt[:],
                        op0=mybir.AluOpType.mult, op1=mybir.AluOpType.max)
                    vt = scpool.tile([128, nch, 8], dt.float32, tag="vt")
                    nc.scalar.activation(vt[:], lt[:],
                                         mybir.ActivationFunctionType.Exp)
                    vb = scpool.tile([128, nch, 8], dt.bfloat16, tag="vb")
                    nc.vector.tensor_copy(vb[:], vt[:])

                    # msg = v * z  (in place over the z part of zg)
                    z4 = zg[:, :, 0:512].rearrange("p c (h d) -> p c h d", d=DH)
                    nc.vector.tensor_tensor(
                        z4, z4, vb[:].to_broadcast([128, nch, 8, DH]),
                        op=mybir.AluOpType.mult)

                    # segment sums on the PE
                    po = p2pool.tile([128, 512], dt.float32, space="PSUM")
                    ps = p2bpool.tile([128, 8], dt.float32, space="PSUM")
                    for ch in range(nch):
                        sl = sel[:, ch * 128:(ch + 1) * 128]
                        nc.tensor.matmul(po[:], sl, zg[:, ch, 0:512],
                                         start=(ch == 0), stop=(ch == nch - 1))
                        nc.tensor.matmul(ps[:], sl, vb[:, ch, :],
                                         start=(ch == 0), stop=(ch == nch - 1))

                    # out = po / s + bias  (eps keeps pad rows finite: 1/eps * 0 = 0)
                    ssb = scpool.tile([128, 8], dt.float32, tag="ssb")
                    nc.vector.tensor_scalar_add(ssb[:], ps[:], 1e-30)
                    nc.vector.reciprocal(ssb[:], ssb[:])
                    osb = eopool.tile([128, 512], dt.float32)
                    o4 = osb[:].rearrange("p (h d) -> p h d", d=DH)
                    nc.vector.tensor_tensor(
                        o4, po[:].rearrange("p (h d) -> p h d", d=DH),
                        ssb[:].to_broadcast([128, 8, DH]),
                        op=mybir.AluOpType.mult)
                    nc.vector.tensor_tensor(osb[:], osb[:], bias_sb[:],
                                            op=mybir.AluOpType.add)
                    nc.sync.dma_start(out_d[t * 128:(t + 1) * 128, :], osb[:])
    nc.compile()
    return nc


# ------------------------------------------------------------------- driver
def kernel(x_src, x_dst, edge_src, edge_dst, W, attn_l, attn_r, bias):
    shared, per_core, k_tile, nch = _host_prep(
        np.asarray(x_src), np.asarray(x_dst), np.asarray(edge_src),
        np.asarray(edge_dst), np.asarray(W), np.asarray(attn_l),
        np.asarray(attn_r), np.asarray(bias))

    nc = _build_nc(k_tile, nch)

    in_maps = []
    for c in range(NCORES):
        in_maps.append({"xT": shared["xT"], "Wext": shared["Wext"],
                        "bias_rep": shared["bias_rep"],
                        "selT": per_core[c]["selT"],
                        "selD": per_core[c]["selD"],
                        "zidx": per_core[c]["zidx"],
                        "eridx": per_core[c]["eridx"]})

    if os.environ.get("KERNEL_SIM"):
        from concourse.bass_interp import CoreSim
        sim = CoreSim(nc, trace=False)
        for name, arr in in_maps[int(os.environ.get("KERNEL_SIM_CORE", "0"))].items():
            sim.tensor(name)[:] = arr
        sim.simulate()
        out = np.array(sim.tensor("out"))
        return np.concatenate([out[:DPC]] * NCORES, 0)  # core-0 slice only

    from concourse.bass_utils import run_bass_kernel_spmd
    res = run_bass_kernel_spmd(nc, in_maps, core_ids=list(range(NCORES)),
                               trace=bool(os.environ.get("KERNEL_TRACE")))
    global LAST_RESULTS
    LAST_RESULTS = res
    return np.concatenate([r["out"][:DPC] for r in res.results], 0)


LAST_RESULTS = None



# revision 31
# speedup vs baseline: 1.1608x; 1.1608x over previous
"""GAT-style bipartite graph attention layer (nn_BiGraphContrastLayer) on 8 trn2 cores.

Strategy (dst-sharded SPMD, one shared program):
  - Each core owns 1250 dst nodes (10 tiles of 128).  Phase 1 computes
    z/el/er = x @ [W | W@Al | W@Ar] for its 10 dst tiles (kept in SBUF) and
    for all 10000 src nodes (79 tiles), writing a src row table
    zel_tab[s] = [z(512) | el(8)] (bf16, 640-elem stride rows) to DRAM.
  - Real edges (no self loops), grouped by dst tile and split into 2 halves
    sorted by src, are gathered per edge via SWDGE dma_gather.  Gather
    descriptor generation is decoupled with prepare_only=True and fired by
    per-half trigger_dma(count=1), so Q7 descriptor emission overlaps phase 1
    and the gather DMA/compute pipeline.
  - v = exp(leaky_relu(el_src + er_dst)) per edge/head (er broadcast to edges
    via one-hot selD matmuls on the PE); msg = v * z_src (DVE); per-dst-tile
    segment sums via one-hot selT matmuls accumulating in PSUM across both
    halves.  Self-loop contribution (v_self, v_self*z_dst) is added from the
    SBUF-resident dst tiles.  Final: (po + self)/(ps + v_self) + bias.
  No inter-core communication; host concatenates the 8 dst slices.
"""
import os

import numpy as np
import ml_dtypes

import concourse.bacc as bacc
import concourse.bass as bass
import concourse.mybir as mybir
import concourse.tile as tile

BF = ml_dtypes.bfloat16
F32 = np.float32

NS, ND, E, DIN, H, DH = 10000, 10000, 320000, 512, 8, 64
NEG = 0.2
NCORES = 8
DPC = ND // NCORES            # 1250 dst nodes per core
NDSTT = 10                    # dst tiles per core
SRCPAD = 10112                # src rows padded to 79 tiles of 128
NSRCT = SRCPAD // 128         # 79
NXT = NSRCT + NDSTT           # 89 xT tiles; 0..78 src, 79..88 dst
ROW = 640                     # zel row stride elems (bf16); 520 used
PANEL = 512                   # phase-1 node panel (4 subtiles of 128)
NXTP = 92                     # xT tiles padded to a PANEL multiple
NHALF = 2 * NDSTT             # 20 gather units per core
ZG_BUFS = 5


# ----------------------------------------------------------------- host prep
def _wrap_idx(idx, k):
    """dma_gather index layout: idx i -> [i % 16, i // 16], replicated 8x.
    idx shorter than k is padded with 0 (row 0 gathered, masked by sel)."""
    full = np.zeros(k, np.int64)
    full[:len(idx)] = idx
    w = np.zeros((16, k // 16), np.int16)
    w[np.arange(k) % 16, np.arange(k) // 16] = full
    return np.tile(w, (8, 1))


def _host_prep(x_src, x_dst, edge_src, edge_dst, W, attn_l, attn_r, bias):
    Al = np.zeros((DIN, H), F32)
    Ar = np.zeros((DIN, H), F32)
    for h in range(H):
        Al[h * DH:(h + 1) * DH, h] = attn_l[h]
        Ar[h * DH:(h + 1) * DH, h] = attn_r[h]
    Wext = np.concatenate([W, W @ Al, W @ Ar], 1).astype(BF)  # [512, 528]
    bias_rep = np.tile(bias[None, :].astype(F32), (128, 1))   # [128, 512]

    edge_src = edge_src.astype(np.int64)
    edge_dst = edge_dst.astype(np.int64)

    # per-(core, dst tile) edge lists sorted by src, split into 2 halves
    halves = [[None] * NHALF for _ in range(NCORES)]
    kmax = 0
    for c in range(NCORES):
        d0 = c * DPC
        m = (edge_dst >= d0) & (edge_dst < d0 + DPC)
        es, ed = edge_src[m], edge_dst[m] - d0
        for t in range(NDSTT):
            sel = (ed >= t * 128) & (ed < (t + 1) * 128)
            ts, td = es[sel], ed[sel] - t * 128
            order = np.argsort(ts, kind="stable")
            ts, td = ts[order], td[order]
            half = (len(ts) + 1) // 2
            halves[c][2 * t] = (ts[:half], td[:half])
            halves[c][2 * t + 1] = (ts[half:], td[half:])
            kmax = max(kmax, half)
    k_half = ((kmax + 127) // 128) * 128
    nch = k_half // 128

    per_core = []
    for c in range(NCORES):
        zidx = np.zeros((128, NHALF * k_half // 16), np.int16)
        selT = np.zeros((128, NHALF * nch * 128), BF)
        selD = np.zeros((128, NHALF * nch * 128), BF)
        for hI in range(NHALF):
            es, edl = halves[c][hI]
            k = len(es)
            s16 = slice(hI * k_half // 16, (hI + 1) * k_half // 16)
            zidx[:, s16] = _wrap_idx(es, k_half)
            dstl = np.full(k_half, -1, np.int64)
            dstl[:k] = edl
            for ch in range(nch):
                dl = dstl[ch * 128:(ch + 1) * 128]
                sm = np.zeros((128, 128), F32)
                valid = dl >= 0
                sm[np.arange(128)[valid], dl[valid]] = 1.0
                j = (hI * nch + ch) * 128
                selT[:, j:j + 128] = sm.astype(BF)
                selD[:, j:j + 128] = sm.T.astype(BF)

        # xT: cols 0..9999 src nodes, cols 10112.. own dst slice
        xc = np.zeros((NXTP * 128, DIN), F32)
        xc[:NS] = x_src
        xc[SRCPAD:SRCPAD + DPC] = x_dst[c * DPC:(c + 1) * DPC]
        xT = np.ascontiguousarray(xc.T).astype(BF)            # [512, 11392]
        per_core.append(dict(xT=xT, selT=selT, selD=selD, zidx=zidx))

    shared = dict(Wext=Wext, bias_rep=bias_rep)
    return shared, per_core, k_half, nch


# ------------------------------------------------------------- bass program
def _build_nc(k_half, nch):
    nc = bacc.Bacc("TRN2", target_bir_lowering=False, debug=False)
    dt = mybir.dt

    xT_d = nc.dram_tensor("xT", [DIN, NXTP * 128], dt.bfloat16,
                          kind="ExternalInput")
    W_d = nc.dram_tensor("Wext", [DIN, 528], dt.bfloat16, kind="ExternalInput")
    bias_d = nc.dram_tensor("bias_rep", [128, 512], dt.float32,
                            kind="ExternalInput")
    selT_d = nc.dram_tensor("selT", [128, NHALF * nch * 128], dt.bfloat16,
                            kind="ExternalInput")
    selD_d = nc.dram_tensor("selD", [128, NHALF * nch * 128], dt.bfloat16,
                            kind="ExternalInput")
    zidx_d = nc.dram_tensor("zidx", [128, NHALF * k_half // 16], dt.int16,
                            kind="ExternalInput")
    out_d = nc.dram_tensor("out", [NDSTT * 128, 512], dt.float32,
                           kind="ExternalOutput")
    zel_d = nc.dram_tensor("zel_tab", [SRCPAD, ROW], dt.bfloat16)

    with tile.TileContext(nc) as tc:
        with tc.tile_pool(name="const", bufs=1) as cpool:
            wsb = cpool.tile([128, 4 * 528], dt.bfloat16)
            for k in range(4):
                nc.sync.dma_start(wsb[:, k * 528:(k + 1) * 528],
                                  W_d[k * 128:(k + 1) * 128, :])
            bias_sb = cpool.tile([128, 512], dt.float32)
            nc.sync.dma_start(bias_sb[:], bias_d[:])
            zidx_sb = cpool.tile([128, NHALF * k_half // 16], dt.int16)
            nc.sync.dma_start(zidx_sb[:], zidx_d[:])
            zdst = cpool.tile([128, NDSTT, 528], dt.bfloat16)
            vself = cpool.tile([128, NDSTT, 8], dt.float32)

            with tc.tile_pool(name="zg", bufs=ZG_BUFS) as zgpool:
                zgt = [zgpool.tile([128, nch, ROW], dt.bfloat16,
                                   tag="zg", name=f"zg{i}")
                       for i in range(NHALF)]

                # ---- phase 1: z/el for src tiles (DRAM zel_tab) + z/el/er
                # for dst tiles (SBUF).  Tiles 0..78 src, 79..88 dst.
                ph1 = tc.tile_pool(name="xp", bufs=2)
                xpool = ph1.__enter__()
                # zel write buffers: full 640-elem rows with the 520:640 tail
                # zeroed once, so zel_tab holds no garbage (gathers read whole
                # rows).
                zel_bufs = [cpool.tile([128, ROW], dt.bfloat16,
                                       name=f"zelbuf{i}") for i in range(3)]
                for zb in zel_bufs:
                    nc.vector.memset(zb[:, 520:ROW], 0)
                ph1c = tc.tile_pool(name="p1", bufs=2, space="PSUM")
                p1pool = ph1c.__enter__()
                ph1d = tc.tile_pool(name="p1b", bufs=2, space="PSUM")
                p1bpool = ph1d.__enter__()
                for p in range(NXTP * 128 // PANEL):
                    xp = xpool.tile([128, 4 * PANEL], dt.bfloat16)
                    for k in range(4):
                        nc.sync.dma_start(
                            xp[:, k * PANEL:(k + 1) * PANEL],
                            xT_d[k * 128:(k + 1) * 128,
                                 p * PANEL:(p + 1) * PANEL])
                    for m in range(PANEL // 128):
                        gt = p * (PANEL // 128) + m   # global tile index
                        if gt >= NXT:
                            continue
                        zps = p1pool.tile([128, 512], dt.float32, space="PSUM")
                        lps = p1bpool.tile([128, 16], dt.float32, space="PSUM")
                        for k in range(4):
                            lhsT = xp[:, k * PANEL + m * 128:
                                      k * PANEL + (m + 1) * 128]
                            nc.tensor.matmul(zps[:], lhsT,
                                             wsb[:, k * 528:k * 528 + 512],
                                             start=(k == 0), stop=(k == 3))
                            nc.tensor.matmul(lps[:], lhsT,
                                             wsb[:, k * 528 + 512:(k + 1) * 528],
                                             start=(k == 0), stop=(k == 3))
                        if gt >= NSRCT:
                            # dst tile: keep z|el|er in SBUF
                            dst_t = gt - NSRCT
                            if gt % 2 == 0:
                                nc.vector.tensor_copy(zdst[:, dst_t, 0:512],
                                                      zps[:])
                                nc.scalar.activation(
                                    zdst[:, dst_t, 512:528], lps[:],
                                    mybir.ActivationFunctionType.Copy)
                            else:
                                nc.scalar.activation(
                                    zdst[:, dst_t, 0:512], zps[:],
                                    mybir.ActivationFunctionType.Copy)
                                nc.vector.tensor_copy(zdst[:, dst_t, 512:528],
                                                      lps[:])
                        else:
                            st = gt                   # src tile index
                            zel_sb = zel_bufs[st % 3]
                            if gt % 2 == 0:
                                nc.vector.tensor_copy(zel_sb[:, 0:512], zps[:])
                                nc.scalar.activation(
                                    zel_sb[:, 512:520], lps[:, 0:8],
                                    mybir.ActivationFunctionType.Copy)
                            else:
                                nc.scalar.activation(
                                    zel_sb[:, 0:512], zps[:],
                                    mybir.ActivationFunctionType.Copy)
                                nc.vector.tensor_copy(zel_sb[:, 512:520],
                                                      lps[:, 0:8])
                            row0 = st * 128
                            nc.sync.dma_start(zel_d[row0:row0 + 128, :],
                                              zel_sb[:])

                ph1d.__exit__(None, None, None)
                ph1c.__exit__(None, None, None)
                ph1.__exit__(None, None, None)

                # vself[d, t, h] = exp(lrelu(el + er)) for the dst self loops
                vtmp = cpool.tile([128, NDSTT, 8], dt.float32)
                nc.vector.tensor_tensor(vtmp[:], zdst[:, :, 512:520],
                                        zdst[:, :, 520:528],
                                        op=mybir.AluOpType.add)
                nc.vector.scalar_tensor_tensor(
                    vtmp[:], vtmp[:], NEG, vtmp[:],
                    op0=mybir.AluOpType.mult, op1=mybir.AluOpType.max)
                nc.scalar.activation(vself[:], vtmp[:],
                                     mybir.ActivationFunctionType.Exp)

                # ---- phase 2: per half-tile gather + attention + aggregation
                with (
                    tc.tile_pool(name="sel", bufs=2) as selpool,
                    tc.tile_pool(name="sc", bufs=3) as scpool,
                    tc.tile_pool(name="eo", bufs=2) as eopool,
                    tc.tile_pool(name="p2", bufs=2, space="PSUM") as p2pool,
                    tc.tile_pool(name="p2b", bufs=2, space="PSUM") as p2bpool,
                    tc.tile_pool(name="p2c", bufs=2, space="PSUM") as p2cpool,
                ):
                    po = ps = None
                    for hI in range(NHALF):
                        t, sub = divmod(hI, 2)
                        sel = selpool.tile([128, nch * 128], dt.bfloat16,
                                           tag="sel")
                        nc.sync.dma_start(
                            sel[:],
                            selT_d[:, hI * nch * 128:(hI + 1) * nch * 128])
                        seld = selpool.tile([128, nch * 128], dt.bfloat16,
                                            tag="seld")
                        nc.sync.dma_start(
                            seld[:],
                            selD_d[:, hI * nch * 128:(hI + 1) * nch * 128])

                        # per-edge gather of [z | el] rows for this half
                        i16 = slice(hI * k_half // 16, (hI + 1) * k_half // 16)
                        nc.gpsimd.dma_gather(
                            zgt[hI][:], zel_d[:], zidx_sb[:, i16],
                            num_idxs=k_half, num_idxs_reg=k_half,
                            elem_size=ROW, single_packet=False)

                        # er broadcast to edges via selD matmuls
                        pe_er = p2cpool.tile([128, nch, 8], dt.float32,
                                             space="PSUM")
                        for ch in range(nch):
                            nc.tensor.matmul(pe_er[:, ch, :],
                                             seld[:, ch * 128:(ch + 1) * 128],
                                             zdst[:, t, 520:528],
                                             start=True, stop=True,
                                             skip_group_check=True)

                        zg = zgt[hI]
                        lt = scpool.tile([128, nch, 8], dt.float32, tag="lt")
                        nc.vector.tensor_tensor(
                            lt[:], zg[:, :, 512:520], pe_er[:],
                            op=mybir.AluOpType.add)
                        nc.vector.scalar_tensor_tensor(
                            lt[:], lt[:], NEG, lt[:],
                            op0=mybir.AluOpType.mult, op1=mybir.AluOpType.max)
                        vb = scpool.tile([128, nch, 8], dt.bfloat16, tag="vb")
                        nc.scalar.activation(vb[:], lt[:],
                                             mybir.ActivationFunctionType.Exp)

                        # msg = v * z  (in place over the z part of zg)
                        z4 = zg[:, :, 0:512].rearrange("p c (h d) -> p c h d",
                                                       d=DH)
                        nc.vector.tensor_tensor(
                            z4, z4, vb[:].to_broadcast([128, nch, 8, DH]),
                            op=mybir.AluOpType.mult)

                        # segment sums accumulated across both halves
                        if sub == 0:
                            po = p2pool.tile([128, 512], dt.float32,
                                             space="PSUM")
                            ps = p2bpool.tile([128, 8], dt.float32,
                                              space="PSUM")
                        for ch in range(nch):
                            sl = sel[:, ch * 128:(ch + 1) * 128]
                            first = sub == 0 and ch == 0
                            last = sub == 1 and ch == nch - 1
                            nc.tensor.matmul(po[:], sl, zg[:, ch, 0:512],
                                             start=first, stop=last)
                            nc.tensor.matmul(ps[:], sl, vb[:, ch, :],
                                             start=first, stop=last)

                        if sub == 0:
                            continue

                        # out = (po + vself*z_dst) / (ps + vself) + bias
                        ssb = scpool.tile([128, 8], dt.float32, tag="ssb")
                        nc.vector.tensor_tensor(ssb[:], ps[:], vself[:, t, :],
                                                op=mybir.AluOpType.add)
                        nc.vector.reciprocal(ssb[:], ssb[:])
                        msf = scpool.tile([128, 512], dt.float32, tag="msf")
                        m4 = msf[:].rearrange("p (h d) -> p h d", d=DH)
                        nc.vector.tensor_tensor(
                            m4, zdst[:, t, 0:512].rearrange(
                                "p (h d) -> p h d", d=DH),
                            vself[:, t, :].to_broadcast([128, 8, DH]),
                            op=mybir.AluOpType.mult)
                        osb = eopool.tile([128, 512], dt.float32)
                        nc.vector.tensor_tensor(osb[:], po[:], msf[:],
                                                op=mybir.AluOpType.add)
                        o4 = osb[:].rearrange("p (h d) -> p h d", d=DH)
                        nc.vector.tensor_tensor(
                            o4, o4, ssb[:].to_broadcast([128, 8, DH]),
                            op=mybir.AluOpType.mult)
                        nc.vector.tensor_tensor(osb[:], osb[:], bias_sb[:],
                                                op=mybir.AluOpType.add)
                        nc.sync.dma_start(out_d[t * 128:(t + 1) * 128, :],
                                          osb[:])
    nc.compile()
    return nc


# ------------------------------------------------------------------- driver
def kernel(x_src, x_dst, edge_src, edge_dst, W, attn_l, attn_r, bias):
    shared, per_core, k_half, nch = _host_prep(
        np.asarray(x_src), np.asarray(x_dst), np.asarray(edge_src),
        np.asarray(edge_dst), np.asarray(W), np.asarray(attn_l),
        np.asarray(attn_r), np.asarray(bias))

    nc = _build_nc(k_half, nch)

    in_maps = []
    for c in range(NCORES):
        in_maps.append({"xT": per_core[c]["xT"], "Wext": shared["Wext"],
                        "bias_rep": shared["bias_rep"],
                        "selT": per_core[c]["selT"],
                        "selD": per_core[c]["selD"],
                        "zidx": per_core[c]["zidx"]})

    if os.environ.get("KERNEL_SIM"):
        from concourse.bass_interp import CoreSim
        sim = CoreSim(nc, trace=False)
        for name, arr in in_maps[int(os.environ.get("KERNEL_SIM_CORE", "0"))].items():
            sim.tensor(name)[:] = arr
        sim.simulate()
        out = np.array(sim.tensor("out"))
        return np.concatenate([out[:DPC]] * NCORES, 0)  # core-0 slice only

    from concourse.bass_utils import run_bass_kernel_spmd
    res = run_bass_kernel_spmd(nc, in_maps, core_ids=list(range(NCORES)),
                               trace=bool(os.environ.get("KERNEL_TRACE")))
    global LAST_RESULTS
    LAST_RESULTS = res
    return np.concatenate([r["out"][:DPC] for r in res.results], 0)


LAST_RESULTS = None
